# revision 23
# baseline (speedup 1.0000x reference)
"""Trainium2 Bass kernel for nn_ColumnStep (scatter_memory).

Contract: kernel(**inputs) takes FULL unsharded inputs (numpy-convertible),
returns the FULL (B, T, V) float32 output.

Sharding: 8 cores = B(2) x T-query-chunks(4); parameters replicated. Host
does only the vocab gather / zero-scatter and layout prep.

Key idea: decay = sigmoid(decay_logit) makes the anti-causal attention
weights decay^(j-i-1) negligible beyond a ~256-token future window
(decay^256 ~ 4e-6 at logit 3.0), so each core only loads/computes a
(512 own + 128*(ND-1) future)-column window instead of the full T=2048
sequence, and the (T x 512) decay-weight DMA collapses to one
[128, 128*ND] Toeplitz band master. Score/retrieve matmuls use bf16
moving operands (full-rate at 128-wide on the PE cost model); projections
stay float32r. All layouts are k-major so rmsnorm reductions are
ones-vector matmuls and no transposes are needed.
"""

import sys

for _p in ("/opt/trn_rl_repo", "/root/.axon_site/_ro/trn_rl_repo"):
    if _p not in sys.path:
        sys.path.append(_p)

import math

import numpy as np

import concourse.bass as bass  # noqa: F401  (registers engine mixins)
import concourse.mybir as mybir
from concourse import bacc, tile
from concourse.bass_utils import run_bass_kernel_spmd

F32 = mybir.dt.float32
F32R = mybir.dt.float32r
BF16 = mybir.dt.bfloat16
AF = mybir.ActivationFunctionType
OP = mybir.AluOpType

# Problem shape (hardcoded per spec)
V, K, B, T, NB, INNER = 32000, 256, 2, 2048, 4, 128
EPS = 1.1920929e-07
P = 128          # partitions
QF = T // 4      # 512 query rows per core
NQ = QF // P     # 4 query tiles per core
KT = K // P      # 2 tiles along the k=256 dim

# pack offsets (f32 columns per partition)
O_W = 0
O_BD = O_W + 4 * KT * K          # 2048
O_BU = O_BD + NB * KT * INNER    # 3072
O_GW = O_BU + NB * K             # 4096
PK = O_GW + KT * NB              # 4104
WK, WQ, WV, WO = 0, 1, 2, 3

_prog_cache = {}


def _build_program(s_qk, c_mem, nd):
    """SPMD Bass/Tile program. nd = number of 128-wide j-tile diagonals
    (1 own + nd-1 future) each query tile attends to."""
    nc = bacc.Bacc("TRN2", target_bir_lowering=False, debug=False, num_devices=8)

    WIN = QF + P * (nd - 1)   # key/value window columns per core
    NJ = NQ + nd - 1          # local j tiles
    MW = P * nd               # decay master columns

    gw_d = nc.dram_tensor("gw", [P, KT, WIN], F32, kind="ExternalInput")
    m_d = nc.dram_tensor("m", [P, MW + 1], F32, kind="ExternalInput")
    pack_d = nc.dram_tensor("pack", [P, PK], F32R, kind="ExternalInput")
    # small: onesc | biash | gatebT (col; rows 0..NB-1 hold gate_b)
    small_d = nc.dram_tensor("small", [P, 3], F32R, kind="ExternalInput")
    # onesr cols: [ones(P) | s_out*ones(P)]
    onesr_d = nc.dram_tensor("onesr", [1, 2 * P], F32R, kind="ExternalInput")
    # branch-selector: e[c, n*P+p] = s_out if c == n else 0
    e_d = nc.dram_tensor("esel", [NB, NB * P], F32R, kind="ExternalInput")
    o_d = nc.dram_tensor("o", [NQ, P, K], F32, kind="ExternalOutput")

    AX = mybir.AxisListType.X

    # rms chunks over the window: [(start, end), ...] in <=512 steps
    chunks = [(c, min(c + 512, WIN)) for c in range(0, WIN, 512)]

    with tile.TileContext(nc) as tc:
        with (
            nc.allow_low_precision(reason="bf16 attention operands validated by rel-err test"),
            tc.tile_pool(name="const", bufs=1) as cp,
            tc.tile_pool(name="persist", bufs=1) as pp,
            tc.tile_pool(name="work", bufs=3) as wp,
            tc.tile_pool(name="stat", bufs=4) as sp,
            tc.tile_pool(name="psA", bufs=2, space="PSUM") as psA,
            tc.tile_pool(name="psS", bufs=3, space="PSUM") as psS,
            tc.tile_pool(name="psR", bufs=1, space="PSUM") as psR,
            tc.tile_pool(name="psN", bufs=1, space="PSUM") as psN,
        ):
            # ---- constants / parameters ----
            pack_t = cp.tile([P, PK], F32R, tag="pack")
            w_t = pack_t[:, O_W:O_BD].rearrange("p (w t k) -> p w t k", w=4, t=KT)
            bd_t = pack_t[:, O_BD:O_BU].rearrange("p (n t h) -> p n t h", n=NB, t=KT)
            bu_t = pack_t[:, O_BU:O_GW].rearrange("p (n k) -> p n k", n=NB)
            gw_wt = pack_t[:, O_GW:PK].rearrange("p (t n) -> p t n", t=KT)
            small_t = cp.tile([P, 3], F32R, tag="small")
            ones_col = small_t[:, 0:1]
            biash_t = small_t[:, 1:2]
            onesr_t = cp.tile([1, 2 * P], F32R, tag="onesr")
            e_t = cp.tile([NB, NB * P], F32R, tag="esel")
            m_t = cp.tile([P, MW + 1], F32, tag="mmat")
            gatebT = m_t[:, MW:MW + 1]  # f32 column (tensor_scalar needs f32)
            eps1_t = cp.tile([1, 1], F32, tag="eps1")
            warm_t = cp.tile([1, 1], F32, tag="warm")
            gw_sb = cp.tile([P, KT, WIN], F32, tag="gwin")

            # ---- act-table warm-up: ONE table slot — warm only Sqrt (the
            # first function used); Exp/Gelu load late, hidden behind PE ----
            nc.vector.memset(eps1_t[:], EPS)
            nc.vector.memset(warm_t[:], 0.0)
            nc.scalar.activation(warm_t[:], warm_t[:], AF.Sqrt)

            # ---- DMAs in priority order (first-use first) ----
            nc.sync.dma_start(gw_sb[:, 0, 0:512], gw_d[:, 0, 0:512])
            nc.sync.dma_start(gw_sb[:, 1, 0:512], gw_d[:, 1, 0:512])
            nc.sync.dma_start(small_t[:], small_d[:])
            nc.sync.dma_start(onesr_t[:], onesr_d[:])
            nc.sync.dma_start(pack_t[:, 0:1024], pack_d[:, 0:1024])        # Wk,Wq
            if WIN > 512:
                nc.sync.dma_start(gw_sb[:, :, 512:WIN], gw_d[:, :, 512:WIN])
            nc.sync.dma_start(m_t[:], m_d[:])
            nc.sync.dma_start(pack_t[:, 1024:2048], pack_d[:, 1024:2048])  # Wv,Wo
            nc.sync.dma_start(e_t[:], e_d[:])
            nc.sync.dma_start(pack_t[:, 2048:3072], pack_d[:, 2048:3072])  # bd
            nc.sync.dma_start(pack_t[:, 3072:PK], pack_d[:, 3072:PK])      # bu,gw

            # ---- persistent intermediates ----
            gnT = [pp.tile([P, WIN], F32R, tag=f"gnT{i}", name=f"gnT{i}") for i in range(KT)]
            kkb = [pp.tile([P, WIN], BF16, tag=f"kkb{i}", name=f"kkb{i}") for i in range(KT)]
            qb = [pp.tile([P, QF], BF16, tag=f"qb{i}", name=f"qb{i}") for i in range(KT)]
            vvb = [pp.tile([P, K], BF16, tag=f"vvb{j}", name=f"vvb{j}") for j in range(NJ)]
            retr_sb = [pp.tile([P, QF], F32R, tag=f"retr{i}", name=f"retr{i}") for i in range(KT)]
            g2T = [pp.tile([P, QF], F32, tag=f"g2T{i}", name=f"g2T{i}") for i in range(KT)]
            gn2T = [pp.tile([P, QF], F32R, tag=f"gn2T{i}", name=f"gn2T{i}") for i in range(KT)]
            hgel = [pp.tile([P, QF], F32R, tag=f"hgel{n}", name=f"hgel{n}") for n in range(NB)]
            hg = [pp.tile([P, QF], F32R, tag=f"hg{n}", name=f"hg{n}") for n in range(NB)]
            exr = pp.tile([NB, QF], F32R, tag="exr")
            o_sb = [pp.tile([P, K], F32, tag=f"o{q}", name=f"o{q}") for q in range(NQ)]

            # ---- rmsnorm (k-major): reduce over partitions via ones matmul;
            # engines for the two squares / two applies are picked to run in
            # parallel (Act + DVE) ----
            def rms_norm(src, dst, c0, c1):
                w = c1 - c0
                sq = wp.tile([P, KT, 512], F32R, tag="sq")
                nc.scalar.square(sq[:, 0, :w], src(0))
                nc.vector.tensor_mul(sq[:, 1, :w], src(1), src(1))
                cs = psN.tile([1, 512], F32, tag="nrm")
                for ki in range(KT):
                    nc.tensor.matmul(cs[:1, :w], ones_col, sq[:, ki, :w],
                                     start=(ki == 0), stop=(ki == KT - 1))
                rt = sp.tile([1, 512], F32R, tag="rt")
                nc.scalar.activation(rt[:1, :w], cs[:1, :w], AF.Sqrt,
                                     bias=eps1_t[:], scale=1.0 / K)
                bc = psN.tile([P, 512], F32, tag="nrm")
                nc.tensor.matmul(bc[:, :w], onesr_t[:, 0:P], rt[:1, :w],
                                 start=True, stop=True)
                rinv = wp.tile([P, 512], F32, tag="rinv")
                nc.vector.reciprocal(rinv[:, :w], bc[:, :w])
                nc.vector.tensor_mul(dst[0][:, c0:c1], src(0), rinv[:, :w])
                nc.gpsimd.tensor_mul(dst[1][:, c0:c1], src(1), rinv[:, :w])

            # ---- pipelined: rmsnorm -> projections -> attention per 512-col
            # chunk; retrieval PSUM is split per 256-col output half so the
            # epilogue for query tiles 0-1 starts while tiles 2-3 attend ----
            retr_ps = [psR.tile([P, QF], F32, tag=f"rps{kt}",
                                name=f"rps{kt}") for kt in range(KT)]

            def attention(jt):
                lo = max(0, jt - (nd - 1))
                hi = min(NQ - 1, jt)
                ib = lo * P
                wdt = (hi - lo + 1) * P
                ms = P * (nd - 1) - P * min(jt, nd - 1)
                sc = psS.tile([P, 512], F32, tag="sc")
                for ki in range(KT):
                    nc.tensor.matmul(
                        sc[:, :wdt], kkb[ki][:, jt * P:(jt + 1) * P],
                        qb[ki][:, ib:ib + wdt],
                        start=(ki == 0), stop=(ki == KT - 1))
                ws = wp.tile([P, 512], BF16, tag="ws")
                nc.vector.tensor_mul(ws[:, :wdt], sc[:, :wdt],
                                     m_t[:, ms:ms + wdt])
                for qt in range(lo, hi + 1):
                    off = qt * P - ib
                    for kt in range(KT):
                        nc.tensor.matmul(
                            retr_ps[kt][:, qt * P:(qt + 1) * P],
                            vvb[jt][:, kt * P:(kt + 1) * P],
                            ws[:, off:off + P],
                            start=(jt == qt), stop=(jt == qt + nd - 1))

            for ci, (c0, c1) in enumerate(chunks):
                w = c1 - c0
                rms_norm(lambda ki, a=c0, b=c1: gw_sb[:, ki, a:b], gnT, c0, c1)
                for ko in range(KT):
                    ps = psA.tile([P, 512], F32, tag="mm")
                    for ki in range(KT):
                        nc.tensor.matmul(
                            ps[:, :w], w_t[:, WK, ki, ko * P:(ko + 1) * P],
                            gnT[ki][:, c0:c1],
                            start=(ki == 0), stop=(ki == KT - 1))
                    nc.scalar.copy(kkb[ko][:, c0:c1], ps[:, :w])
                if ci == 0:
                    for ko in range(KT):
                        ps = psA.tile([P, 512], F32, tag="mm")
                        for ki in range(KT):
                            nc.tensor.matmul(
                                ps[:], w_t[:, WQ, ki, ko * P:(ko + 1) * P],
                                gnT[ki][:, 0:QF],
                                start=(ki == 0), stop=(ki == KT - 1))
                        nc.vector.tensor_scalar(qb[ko][:], ps[:], s_qk, None,
                                                op0=OP.mult)
                for jt in range(c0 // P, min(c1 // P, NJ)):
                    ps = psA.tile([P, K], F32, tag="mm")
                    for ki in range(KT):
                        nc.tensor.matmul(
                            ps[:], gnT[ki][:, jt * P:(jt + 1) * P], w_t[:, WV, ki, :],
                            start=(ki == 0), stop=(ki == KT - 1))
                    if jt % 2 == 0:
                        nc.scalar.copy(vvb[jt][:], ps[:])
                    else:
                        nc.vector.tensor_copy(vvb[jt][:], ps[:])
                for jt in range(c0 // P, min(c1 // P, NJ)):
                    attention(jt)

            # ---- epilogue per 256-col half (query tiles 2h, 2h+1):
            # Wo -> residual -> rmsnorm -> gates/exp -> MLP -> gated up-proj
            # -> per-partition softmax normalize at the output copy ----
            def epilogue(h):
                hc = slice(2 * h * P, 2 * (h + 1) * P)
                nc.scalar.copy(retr_sb[0][:, hc], retr_ps[0][:, hc])
                nc.vector.tensor_copy(retr_sb[1][:, hc], retr_ps[1][:, hc])
                for ko in range(KT):
                    ps = psA.tile([P, 2 * P], F32, tag="mm")
                    for ki in range(KT):
                        nc.tensor.matmul(
                            ps[:], w_t[:, WO, ki, ko * P:(ko + 1) * P],
                            retr_sb[ki][:, hc],
                            start=(ki == 0), stop=(ki == KT - 1))
                    nc.vector.scalar_tensor_tensor(
                        g2T[ko][:, hc], ps[:], c_mem, gw_sb[:, ko, hc],
                        op0=OP.mult, op1=OP.add)
                rms_norm(lambda ki: g2T[ki][:, hc], gn2T,
                         2 * h * P, 2 * (h + 1) * P)
                gp = psS.tile([NB, 2 * P], F32, tag="sc")
                for ki in range(KT):
                    nc.tensor.matmul(gp[:], gw_wt[:, ki, :], gn2T[ki][:, hc],
                                     start=(ki == 0), stop=(ki == KT - 1))
                # exp(logits + gate_b): gate_b folded into the activation bias
                nc.scalar.activation(exr[:, hc], gp[:], AF.Exp,
                                     bias=gatebT[0:NB, :])
                rcT = []
                for qt in (2 * h, 2 * h + 1):
                    smT = psN.tile([P, 2], F32, tag="nrm")
                    nc.tensor.matmul(smT[:], exr[:, qt * P:(qt + 1) * P],
                                     small_t[0:NB, 0:2], start=True, stop=True)
                    rc = sp.tile([P, 1], F32, tag="rcT")
                    nc.vector.reciprocal(rc[:], smT[:, 0:1])
                    rcT.append(rc)
                for n in range(NB):
                    hp = psA.tile([P, 2 * P], F32, tag="mm")
                    for ki in range(KT):
                        nc.tensor.matmul(
                            hp[:], bd_t[:, n, ki, :], gn2T[ki][:, hc],
                            start=(ki == 0), stop=(ki == KT - 1))
                    nc.scalar.activation(hgel[n][:, hc], hp[:], AF.Gelu,
                                         bias=biash_t)
                # gate with UNNORMALIZED s_out*exp gates (normalize at output)
                for n in range(NB):
                    gb = psS.tile([P, 2 * P], F32, tag="sc")
                    nc.tensor.matmul(gb[:], e_t[:, n * P:(n + 1) * P],
                                     exr[:, hc], start=True, stop=True)
                    nc.vector.tensor_mul(hg[n][:, hc], hgel[n][:, hc], gb[:])
                if h == 0:
                    bp = [psS.tile([P, K], F32, tag="sc", name=f"bp0{kt}")
                          for kt in range(KT)]
                else:
                    bp = [psR.tile([P, K], F32, tag=f"rps{kt}",
                                   name=f"bp1{kt}") for kt in range(KT)]
                for n in range(NB):
                    for ii, qt in enumerate((2 * h, 2 * h + 1)):
                        nc.tensor.matmul(
                            bp[ii][:], hg[n][:, qt * P:(qt + 1) * P],
                            bu_t[:, n, :],
                            start=(n == 0), stop=(n == NB - 1))
                for ii, qt in enumerate((2 * h, 2 * h + 1)):
                    nc.scalar.activation(o_sb[qt][:], bp[ii][:], AF.Copy,
                                         scale=rcT[ii][:])
                    nc.sync.dma_start(o_d[qt], o_sb[qt][:])

            epilogue(0)
            epilogue(1)

    nc.compile()
    return nc


def kernel(**inputs):
    x = np.asarray(inputs["x"], np.float32)
    Wq = np.asarray(inputs["Wq"], np.float32)
    Wk = np.asarray(inputs["Wk"], np.float32)
    Wv = np.asarray(inputs["Wv"], np.float32)
    Wo = np.asarray(inputs["Wo"], np.float32)
    decay_logit = np.float32(np.asarray(inputs["decay_logit"]).reshape(()))
    out_scale = np.float32(np.asarray(inputs["out_scale"]).reshape(()))
    mem_scale = np.float32(np.asarray(inputs["mem_scale"]).reshape(-1)[0])
    branch_down = np.asarray(inputs["branch_down"], np.float32)
    branch_up = np.asarray(inputs["branch_up"], np.float32)
    mlp_bias = np.asarray(inputs["mlp_bias"], np.float32)
    gate_W = np.asarray(inputs["gate_W"], np.float32)
    gate_b = np.asarray(inputs["gate_b"], np.float32)
    write_scale = np.float32(np.asarray(inputs["write_scale"]).reshape(()))
    read_idx = np.asarray(inputs["read_indices"]).astype(np.int64)
    write_idx = np.asarray(inputs["write_indices"]).astype(np.int64)

    # Host-side gather of the active vocab subspace (data movement only).
    g = np.take(x, read_idx, axis=2)  # (B, T, K)

    decay = float(1.0 / (1.0 + np.exp(-float(decay_logit))))
    # window depth: smallest nd with decay^(128*(nd-1)) <= 3e-5 (first
    # omitted diagonal's largest weight); nd=2 minimum, 16 = full sequence
    if decay <= 0.0:
        nd = 2
    else:
        nd = max(2, 1 + int(math.ceil(math.log(3e-5) / math.log(decay) / 128.0)))
    nd = min(nd, 16)

    s_qk = float(1.0 / np.sqrt(np.float32(K)))
    c_mem = float(out_scale * mem_scale)
    s_out = float(write_scale * np.float32(1.0 / 16.0))

    key = (round(s_qk, 12), round(c_mem, 12), nd)
    nc = _prog_cache.get(key)
    if nc is None:
        nc = _build_program(s_qk, c_mem, nd)
        _prog_cache[key] = nc

    WIN = QF + P * (nd - 1)
    MW = P * nd

    # Replicated parameter pack (partition-first); wall order [Wk,Wq,Wv,Wo].
    wall = np.stack([Wk, Wq, Wv, Wo]).reshape(4, KT, P, K).transpose(2, 0, 1, 3)
    bdall = branch_down.reshape(NB, KT, P, INNER).transpose(2, 0, 1, 3)
    buall = branch_up.transpose(1, 0, 2)
    gwp = gate_W.reshape(KT, P, NB).transpose(1, 0, 2)
    pack = np.concatenate([
        wall.reshape(P, -1), bdall.reshape(P, -1), buall.reshape(P, -1),
        gwp.reshape(P, -1)], axis=1).astype(np.float32)
    small = np.zeros((P, 3), np.float32)
    small[:, 0] = 1.0
    small[:, 1] = mlp_bias
    onesr = np.ones((1, 2 * P), np.float32)
    onesr[0, P:] = s_out
    esel = np.zeros((NB, NB * P), np.float32)
    for _n in range(NB):
        esel[_n, _n * P:(_n + 1) * P] = s_out

    # Toeplitz decay master: M[jl, m] = decay^(128*(nd-1) + jl - m - 1),
    # zero where the exponent would be negative (j <= i).
    jl = np.arange(P, dtype=np.float64)[:, None]
    mm = np.arange(MW, dtype=np.float64)[None, :]
    e = P * (nd - 1) + jl - mm - 1.0
    M = np.where(e >= 0, np.power(decay, np.maximum(e, 0.0)), 0.0).astype(np.float32)
    M = np.concatenate([M, np.zeros((P, 1), np.float32)], axis=1)
    M[:NB, MW] = gate_b

    in_maps = []
    for c in range(8):
        b, qc = divmod(c, NQ)
        c0 = qc * QF
        navail = min(WIN, T - c0)
        win = np.zeros((WIN, K), np.float32)
        win[:navail] = g[b][c0:c0 + navail]
        gwc = np.ascontiguousarray(
            win.T.reshape(KT, P, WIN).transpose(1, 0, 2))
        in_maps.append({
            "gw": gwc, "m": M, "pack": pack, "small": small, "onesr": onesr,
            "esel": esel,
        })

    res = run_bass_kernel_spmd(nc, in_maps, list(range(8)))

    out = np.zeros((B, T, V), np.float32)
    for c in range(8):
        b, qc = divmod(c, NQ)
        oc = res.results[c]["o"].reshape(QF, K)
        out[b, qc * QF:(qc + 1) * QF, :][:, write_idx] = oc
    return out
